# revision 36
# baseline (speedup 1.0000x reference)
"""Trainium2 Bass kernel for nn_AMIPRouterInference (gnn_message_passing).

Strategy
--------
Algebraic restructure of the reference (~515 GFLOP -> ~52 GFLOP):
  * cond @ W1 splits into h_anc @ W1a + h_ctr @ W1b, each computed once per
    token (not once per window pair):  u = h @ W1b, v = h @ W1a.
  * The attention combine over the +-r window commutes with the W2 matmul:
    g = sum_n cw_n * gelu(v[l+off_n] + u[l]);  delta = (w * g) @ W2 + w @ b2.

Sharding: pure data-parallel over the B*L = 4096 tokens -> 512 tokens/core on
8 cores; the +-5 halo is baked into each core's input shard on the host, so no
collectives are needed.

Per-core layout: features-on-partitions (u/v as 16 chunks of [128, tokens]) so
window shifts along tokens are free-axis SBUF slices.  Even/odd phase copies of
v keep the bf16 DVE 2x alignment for shifted adds.

Engine/queue discipline (all queues are in-order!):
  * PE queue: gram/router MMs -> u/v MMs fc0.. -> cw/w transposes -> more u/v
    MMs -> delta MMs.  Nothing in PE's queue ever waits on the softmax chain
    before matmul work is exhausted.
  * Sync DMA queue: h + small consts -> per-fc W1 streams -> W2 (stage E only)
    -> output stores.  The broadcast DMAs live on the GPSIMD queue, which has
    nothing else to do until the first combine (which needs them anyway).
  * DVE: batched window adds (overlapping-window APs), combine tree; ACT: all
    PSUM evacuation + gelu; GPSIMD: a slice of the cw multiply.
"""

import sys

for _p in ("/opt/trn_rl_repo", "/root/.axon_site/_ro/trn_rl_repo"):
    if _p not in sys.path:
        sys.path.append(_p)

import numpy as np
import ml_dtypes

import bass_rust
import concourse.bacc as bacc
import concourse.mybir as mybir
import concourse.tile as tile
from concourse.bass_utils import run_bass_kernel_spmd

BF16 = ml_dtypes.bfloat16

# Problem constants (hardcoded per spec).
B, L, D = 2, 2048, 1024
K, D4, R = 8, 256, 5
NCORES = 8
T = (B * L) // NCORES          # tokens per core = 512
PADL = 16                      # left pad of the per-core token window
TP = T + 2 * PADL              # padded width = 544
NOFF = 2 * R                   # 10 window offsets
F = K * D4                     # 2048 fused expert features
NFC = F // 128                 # 16 feature chunks
NKC = D // 128                 # 8 contraction chunks
NTC = T // 128                 # 4 token tiles per core

# Offset processing order: even offsets first (read from v_even), then odd
# (read from v_odd, which holds v shifted left by one token).  Within each
# phase the SBUF slice starts are even element indices -> 4-byte aligned.
OFF_ORDER = [-4, -2, 2, 4, -5, -3, -1, 1, 3, 5]

GPS_SPLIT = 3584               # cw-mult columns on GPSIMD; rest on DVE

_CACHE = {}


def _build_graph():
    fp32 = mybir.dt.float32
    bf16 = mybir.dt.bfloat16

    nc = bacc.Bacc("TRN2", target_bir_lowering=False, debug=False,
                   num_devices=NCORES)

    # ---- DRAM parameters (per-core shards; same shapes on every core) ----
    hT = nc.dram_tensor("hT", [NKC, 128, TP], bf16, kind="ExternalInput")
    w1a = nc.dram_tensor("w1a", [NFC, 128, D], bf16, kind="ExternalInput")
    w1b = nc.dram_tensor("w1b", [NFC, 128, D], bf16, kind="ExternalInput")
    w2 = nc.dram_tensor("w2", [NFC, 128, D], bf16, kind="ExternalInput")
    wr = nc.dram_tensor("wr", [NKC, 128, K], bf16, kind="ExternalInput")
    # validity transposed: [NOFF, T]
    validT = nc.dram_tensor("validT", [NOFF, T], fp32, kind="ExternalInput")
    ones10 = nc.dram_tensor("ones10", [NOFF, 1], bf16, kind="ExternalInput")
    br_bc = nc.dram_tensor("br_bc", [128, NTC * K], fp32, kind="ExternalInput")
    b1s = nc.dram_tensor("b1s", [128, NFC], fp32, kind="ExternalInput")
    b2s = nc.dram_tensor("b2s", [K, D], bf16, kind="ExternalInput")
    ident = nc.dram_tensor("ident", [128, 128], fp32, kind="ExternalInput")
    out = nc.dram_tensor("out", [T, D], fp32, kind="ExternalOutput")

    AF = mybir.ActivationFunctionType
    OP = mybir.AluOpType

    def bc_ap(tile_, inner_rep, ncols):
        """[128, ncols] tile viewed as [128, ncols, inner_rep] via a step-0
        innermost dim (per-partition broadcast along the replicated axis)."""
        return bass_rust.AP(
            tensor=tile_[:].tensor, offset=0,
            ap=[[ncols, 128], [1, ncols], [0, inner_rep]])

    with tile.TileContext(nc) as tc:
        with (
            tc.tile_pool(name="const", bufs=1) as cpool,
            tc.tile_pool(name="hpool", bufs=1) as hpool,
            tc.tile_pool(name="w2pool", bufs=1) as w2pool,
            tc.tile_pool(name="w1pool", bufs=2) as w1pool,
            tc.tile_pool(name="small", bufs=2) as spool,
            tc.tile_pool(name="persist", bufs=1) as ppool,
            tc.tile_pool(name="uv", bufs=2) as uvpool,
            tc.tile_pool(name="big", bufs=3) as bigpool,
            tc.tile_pool(name="qbuf", bufs=1) as q1pool,
            tc.tile_pool(name="tbuf", bufs=1) as qpool,
            tc.tile_pool(name="ppart", bufs=1) as partpool,
            tc.tile_pool(name="gout", bufs=1) as gpool,
            tc.tile_pool(name="opool", bufs=2) as opool,
            tc.tile_pool(name="dram", bufs=1, space="DRAM") as dpool,
            tc.tile_pool(name="ps_big", bufs=4, space="PSUM") as psb,
            tc.tile_pool(name="ps_small", bufs=4, space="PSUM") as pss,
            # NOTE: all psb tiles share tag "m" (4 banks), all pss tiles share
            # tag "s" (4 banks) -- PSUM has only 8 banks total.
        ):
            # ---------------- constant / input loads ----------------
            h_sb = []
            for kc in range(NKC):
                t = hpool.tile([128, TP], bf16, tag=f"h{kc}")
                nc.sync.dma_start(t[:], hT[kc])
                h_sb.append(t)
            wr_sb = []
            for kc in range(NKC):
                t = cpool.tile([128, K], bf16, tag=f"wr{kc}")
                nc.sync.dma_start(t[:], wr[kc])
                wr_sb.append(t)
            ident_sb = cpool.tile([128, 128], fp32, tag="ident")
            nc.sync.dma_start(ident_sb[:], ident[:])
            br_sb = cpool.tile([128, NTC * K], fp32, tag="br")
            nc.sync.dma_start(br_sb[:], br_bc[:])
            b1_sb = cpool.tile([128, NFC], fp32, tag="b1")
            nc.sync.dma_start(b1_sb[:], b1s[:])
            b2_sb = cpool.tile([K, D], bf16, tag="b2")
            nc.sync.dma_start(b2_sb[:], b2s[:])
            validT_sb = cpool.tile([NOFF, T], fp32, tag="validT")
            nc.sync.dma_start(validT_sb[:], validT[:])
            ones10_sb = cpool.tile([NOFF, 1], bf16, tag="ones10")
            nc.sync.dma_start(ones10_sb[:], ones10[:])

            # persistent transposed score & router weights (bf16)
            cwT_bf = ppool.tile([NOFF, T], bf16, tag="cwT")
            wT_bf = ppool.tile([K, T], bf16, tag="wT")
            cw_bc = gpool.tile([128, NOFF * 512], bf16, tag="cw_bc")
            w_bc_all = gpool.tile([128, K * 512], bf16, tag="w_bc_all")

            # ------------- stage A/B/C: scores, cw, router w -------------
            # Phase A: all gram/router matmuls first (PE never blocks).
            gram_ps, logit_ps = [], []
            for tci in range(NTC):
                c0 = PADL + tci * 128
                g_ps = psb.tile([128, 512], fp32, tag="m")
                lg_ps = pss.tile([128, K], fp32, tag="s")
                for kc in range(NKC):
                    st = (kc == 0)
                    sp = (kc == NKC - 1)
                    nc.tensor.matmul(g_ps[:, :138],
                                     h_sb[kc][:, c0:c0 + 128],
                                     h_sb[kc][:, c0 - 5:c0 + 133],
                                     start=st, stop=sp)
                    nc.tensor.matmul(lg_ps[:],
                                     h_sb[kc][:, c0:c0 + 128],
                                     wr_sb[kc][:],
                                     start=st, stop=sp)
                gram_ps.append(g_ps)
                logit_ps.append(lg_ps)

            # Phase B1: diagonal extraction, transpose scores to [NOFF, T],
            # then broadcast the UNNORMALIZED ev = exp(s)*valid right away.
            # The softmax 1/den lands at the very end as a per-token (=per-
            # partition) scale on stage E's output.  No max-shift is needed:
            # scores are O(1) so exp() cannot over/underflow.
            s_all = spool.tile([128, NTC * NOFF], fp32, tag="s_all")
            junk = spool.tile([128, 128], fp32, tag="junk")
            for tci in range(NTC):
                g_ps = gram_ps[tci]
                for n, off in enumerate(OFF_ORDER):
                    nc.vector.affine_mul_reduce(
                        junk[:], s_all[:, tci * NOFF + n:tci * NOFF + n + 1],
                        g_ps[:, off + 5:off + 5 + 128], ident_sb[:],
                        1.0 / 32.0, 0.0)
            # router logits evac first: frees the "s" PSUM banks that the
            # score transposes and stage D's vb_ps need.
            lg_all = spool.tile([128, NTC * K], fp32, tag="lg_all")
            for tci in range(NTC):
                nc.scalar.copy(lg_all[:, tci * K:(tci + 1) * K],
                               logit_ps[tci][:])
            sT = ppool.tile([NOFF, T], fp32, tag="sT")
            for tci in range(NTC):
                sT_ps = pss.tile([NOFF, 128], fp32, tag="s")
                nc.tensor.transpose(sT_ps[:],
                                    s_all[:, tci * NOFF:(tci + 1) * NOFF],
                                    ident_sb[:])
                nc.scalar.copy(sT[:, tci * 128:(tci + 1) * 128], sT_ps[:])
            evT = ppool.tile([NOFF, T], fp32, tag="evT")
            nc.scalar.activation(evT[:], sT[:], AF.Exp)
            nc.vector.tensor_mul(cwT_bf[:], evT[:], validT_sb[:])
            cw_dram = dpool.tile([1, NOFF * T], bf16, tag="cw_dram")
            nc.scalar.dma_start(cw_dram[:], cwT_bf[:])
            nc.scalar.dma_start(cw_bc[:],
                                cw_dram[:].partition_broadcast(128))

            rdenT = ppool.tile([128, NTC], fp32, tag="rdenT")
            fT = ppool.tile([128, NTC], fp32, tag="fT")
            wplT_bf = ppool.tile([K, T], bf16, tag="wplT")

            def phase_b2():
                """Denominators + router softmax + w transposes/broadcast."""
                den_ps = pss.tile([1, T], fp32, tag="s")
                nc.tensor.matmul(den_ps[:], ones10_sb[:], cwT_bf[:],
                                 start=True, stop=True)
                den = ppool.tile([1, T], fp32, tag="den")
                nc.scalar.copy(den[:], den_ps[:])       # raw sum_n ev
                dene = ppool.tile([1, T], fp32, tag="dene")
                nc.vector.tensor_scalar_add(dene[:], den[:], 1e-30)
                rden = ppool.tile([1, T], fp32, tag="rden")
                nc.vector.reciprocal(rden[:], dene[:])
                # per-token columns (tokens -> partitions) via tiny DMAs
                for tci in range(NTC):
                    nc.scalar.dma_start(rdenT[:, tci:tci + 1],
                                        rden[:, tci * 128:(tci + 1) * 128])
                    nc.scalar.dma_start(fT[:, tci:tci + 1],
                                        den[:, tci * 128:(tci + 1) * 128])

                # router softmax (batched over tci)
                nc.vector.tensor_add(lg_all[:], lg_all[:], br_sb[:])
                wmx = spool.tile([128, NTC], fp32, tag="wmx")
                lg3 = bass_rust.AP(tensor=lg_all[:].tensor, offset=0,
                                   ap=[[NTC * K, 128], [K, NTC], [1, K]])
                nc.vector.reduce_max(wmx[:], lg3, mybir.AxisListType.X)
                we = spool.tile([128, NTC * K], fp32, tag="we")
                nc.vector.tensor_sub(we[:], lg_all[:], bc_ap(wmx, K, NTC))
                nc.scalar.activation(we[:], we[:], AF.Exp)
                wsum = spool.tile([128, NTC], fp32, tag="wsum")
                we3 = bass_rust.AP(tensor=we[:].tensor, offset=0,
                                   ap=[[NTC * K, 128], [K, NTC], [1, K]])
                nc.vector.reduce_sum(wsum[:], we3, mybir.AxisListType.X)
                rws = spool.tile([128, NTC], fp32, tag="rws")
                nc.vector.reciprocal(rws[:], wsum[:])
                w_pl = spool.tile([128, NTC * K], fp32, tag="w_pl")
                nc.vector.tensor_mul(w_pl[:], we[:], bc_ap(rws, K, NTC))
                # b2-path weights: w * raw_den (so that the final 1/den scale
                # on stage E's output reproduces w * sum_cw exactly)
                weff3 = spool.tile([128, NTC * K], fp32, tag="weff3")
                for tci in range(NTC):
                    nc.vector.tensor_scalar_mul(
                        weff3[:, tci * K:(tci + 1) * K],
                        w_pl[:, tci * K:(tci + 1) * K], fT[:, tci:tci + 1])
                for tci in range(NTC):
                    wpT_ps = pss.tile([K, 128], fp32, tag="s")
                    nc.tensor.transpose(wpT_ps[:],
                                        w_pl[:, tci * K:(tci + 1) * K],
                                        ident_sb[:])
                    nc.scalar.copy(wplT_bf[:, tci * 128:(tci + 1) * 128],
                                   wpT_ps[:])
                    weT_ps = pss.tile([K, 128], fp32, tag="s")
                    nc.tensor.transpose(weT_ps[:],
                                        weff3[:, tci * K:(tci + 1) * K],
                                        ident_sb[:])
                    nc.scalar.copy(wT_bf[:, tci * 128:(tci + 1) * 128],
                                   weT_ps[:])
                w_dram = dpool.tile([1, K * T], bf16, tag="w_dram")
                nc.scalar.dma_start(w_dram[:], wplT_bf[:])
                nc.scalar.dma_start(w_bc_all[:],
                                    w_dram[:].partition_broadcast(128))

            # ------------- stage D: u/v matmuls + gelu combine -------------
            g_sb = [None] * NFC
            tmp_sb = [None] * NFC

            def stage_d_mm(fc):
                w1b_t = w1pool.tile([128, D], bf16, tag="w1b")
                nc.sync.dma_start(w1b_t[:], w1b[fc])
                w1a_t = w1pool.tile([128, D], bf16, tag="w1a")
                nc.sync.dma_start(w1a_t[:], w1a[fc])

                u_ps = psb.tile([128, 512], fp32, tag="m")
                va_ps = psb.tile([128, 512], fp32, tag="m")
                vb_ps = pss.tile([128, 48], fp32, tag="s")
                for kc in range(NKC):
                    st = (kc == 0)
                    sp = (kc == NKC - 1)
                    lhs_b = w1b_t[:, kc * 128:(kc + 1) * 128]
                    lhs_a = w1a_t[:, kc * 128:(kc + 1) * 128]
                    nc.tensor.matmul(u_ps[:], lhs_b,
                                     h_sb[kc][:, PADL:PADL + 512],
                                     start=st, stop=sp)
                    nc.tensor.matmul(va_ps[:], lhs_a,
                                     h_sb[kc][:, 0:512],
                                     start=st, stop=sp)
                    nc.tensor.matmul(vb_ps[:], lhs_a,
                                     h_sb[kc][:, 496:544],
                                     start=st, stop=sp)

                u_sb = uvpool.tile([128, 512], bf16, tag="u")
                nc.scalar.copy(u_sb[:], u_ps[:])
                v_ev = uvpool.tile([128, TP], bf16, tag="v_ev")
                nc.scalar.copy(v_ev[:, 0:512], va_ps[:])
                nc.scalar.copy(v_ev[:, 512:544], vb_ps[:, 16:48])
                # odd phase built straight from PSUM (keeps DMA out of the
                # critical chain)
                v_od = uvpool.tile([128, TP], bf16, tag="v_od")
                nc.scalar.copy(v_od[:, 0:511], va_ps[:, 1:512])
                nc.scalar.copy(v_od[:, 511:543], vb_ps[:, 16:48])

                tmp = bigpool.tile([128, NOFF * 512], bf16, tag="tmp")
                # 10 contiguous shifted adds -- contiguous 1D free APs keep
                # the DVE in its bf16 2x perf mode (overlapping multi-dim
                # APs drop to 1x).
                for n, off in enumerate(OFF_ORDER):
                    if off % 2 == 0:
                        vsrc = v_ev[:, PADL + off:PADL + off + 512]
                    else:
                        vsrc = v_od[:, PADL - 1 + off:PADL - 1 + off + 512]
                    nc.vector.tensor_add(
                        tmp[:, n * 512:(n + 1) * 512], vsrc, u_sb[:])
                nc.scalar.activation(tmp[:], tmp[:], AF.Gelu,
                                     bias=b1_sb[:, fc:fc + 1])
                tmp_sb[fc] = tmp

            def stage_d_combine(fc):
                tmp = tmp_sb[fc]
                q = q1pool.tile([128, NOFF * 512], bf16, tag="q")
                # NOTE: GPSIMD shares SBUF ports with DVE -- offloading
                # elementwise work there just steals DVE bandwidth, so the
                # whole multiply stays on DVE.
                nc.vector.tensor_mul(q[:], tmp[:], cw_bc[:])

                # pairwise tree-sum of the 10 weighted slices, then w-scale
                t1 = qpool.tile([128, 2560], bf16, tag="t1")
                nc.vector.tensor_add(t1[:], q[:, 0:2560], q[:, 2560:5120])
                t2 = qpool.tile([128, 1024], bf16, tag="t2")
                nc.vector.tensor_add(t2[:], t1[:, 0:1024], t1[:, 1024:2048])
                t3 = qpool.tile([128, 512], bf16, tag="t3")
                nc.vector.tensor_add(t3[:], t2[:, 0:512], t2[:, 512:1024])
                t4 = qpool.tile([128, 512], bf16, tag="t4")
                nc.vector.tensor_add(t4[:], t3[:], t1[:, 2048:2560])
                g_t = gpool.tile([128, 512], bf16, tag=f"g{fc}")
                nc.vector.tensor_mul(
                    g_t[:], t4[:],
                    w_bc_all[:, (fc // 2) * 512:(fc // 2) * 512 + 512])
                g_sb[fc] = g_t

            w2_sb = [None] * NFC

            def load_w2(j):
                t = w2pool.tile([128, D], bf16, tag=f"w2_{j}")
                nc.sync.dma_start(t[:], w2[j])
                w2_sb[j] = t

            d_part = [None] * 8

            def stage_e1(blk):
                """Partial delta over fc 0..7 -- interleaved mid stage-D."""
                tci, dh = blk // 2, blk % 2
                d_ps = pss.tile([128, 512], fp32, tag="s")
                for fc in range(8):
                    nc.tensor.matmul(
                        d_ps[:],
                        g_sb[fc][:, tci * 128:(tci + 1) * 128],
                        w2_sb[fc][:, dh * 512:(dh + 1) * 512],
                        start=(fc == 0), stop=(fc == 7))
                p_t = partpool.tile([128, 512], bf16, tag=f"p{blk}")
                nc.scalar.mul(p_t[:], d_ps[:], rdenT[:, tci:tci + 1])
                d_part[blk] = p_t

            def stage_e2(blk):
                tci, dh = blk // 2, blk % 2
                d_ps = pss.tile([128, 512], fp32, tag="s")
                for fc in range(8, NFC):
                    nc.tensor.matmul(
                        d_ps[:],
                        g_sb[fc][:, tci * 128:(tci + 1) * 128],
                        w2_sb[fc][:, dh * 512:(dh + 1) * 512],
                        start=(fc == 8), stop=False)
                nc.tensor.matmul(
                    d_ps[:],
                    wT_bf[:, tci * 128:(tci + 1) * 128],
                    b2_sb[:, dh * 512:(dh + 1) * 512],
                    start=False, stop=True)
                o_sb = opool.tile([128, 512], fp32, tag="o")
                nc.vector.scalar_tensor_tensor(
                    o_sb[:], d_ps[:], rdenT[:, tci:tci + 1], d_part[blk][:],
                    op0=OP.mult, op1=OP.add)
                nc.sync.dma_start(
                    out[tci * 128:(tci + 1) * 128,
                        dh * 512:(dh + 1) * 512], o_sb[:])

            # ---- emission schedule ----
            # 2-fc runway hides the softmax->broadcast latency; combines then
            # run at lag 1.  E1 (delta over fc 0..7) interleaves into the
            # back half; the tail is combine(15) + E2 only.
            stage_d_mm(0)
            phase_b2()
            stage_d_mm(1)
            stage_d_combine(0)
            e1_done = 0
            for fc in range(2, NFC):
                stage_d_mm(fc)
                stage_d_combine(fc - 1)
                if 2 <= fc < 10:
                    load_w2(fc - 2)         # w2 chunks 0..7 for E1
                    load_w2(fc + 6)         # w2 chunks 8..15 for E2
                if fc >= 11:                # g 0..7 + w2 0..7 ready
                    for _ in range(2):
                        if e1_done < 8:
                            stage_e1(e1_done)
                            e1_done += 1
            stage_d_combine(NFC - 1)
            while e1_done < 8:
                stage_e1(e1_done)
                e1_done += 1
            for blk in range(8):
                stage_e2(blk)

    nc.compile()
    return nc


def _prep_shards(h_L, mask_flags, Wr, br, W1, b1, W2, b2):
    """Host-side shard construction (numpy only; cheap vs device work)."""
    f32 = np.float32
    h_L = np.asarray(h_L, f32)
    mask = np.asarray(mask_flags)
    Wr = np.asarray(Wr, f32)
    W1 = np.asarray(W1, f32)
    W2 = np.asarray(W2, f32)
    br = np.asarray(br, f32)
    b1 = np.asarray(b1, f32)
    b2 = np.asarray(b2, f32)

    # shared (replicated) weight blocks
    w1a = np.ascontiguousarray(
        W1[:, :D, :].transpose(1, 0, 2).reshape(D, F)
        .reshape(NKC, 128, NFC, 128).transpose(2, 1, 0, 3)
        .reshape(NFC, 128, D)).astype(BF16)
    w1b = np.ascontiguousarray(
        W1[:, D:, :].transpose(1, 0, 2).reshape(D, F)
        .reshape(NKC, 128, NFC, 128).transpose(2, 1, 0, 3)
        .reshape(NFC, 128, D)).astype(BF16)
    w2 = np.ascontiguousarray(W2.reshape(F, D).reshape(NFC, 128, D)).astype(BF16)
    wr = np.ascontiguousarray(Wr.reshape(NKC, 128, K)).astype(BF16)
    br_bc = np.tile(np.broadcast_to(br[None, :], (128, K)), (1, NTC)).astype(f32)
    b1s = np.ascontiguousarray(b1.reshape(F).reshape(NFC, 128).T)
    b2s = b2.astype(BF16)
    identm = np.eye(128, dtype=f32)

    offs = np.array(OFF_ORDER, np.int64)
    in_maps = []
    outs_meta = []
    per_batch = L // (NCORES // B)          # 512 tokens, 4 shards per batch
    for c in range(NCORES):
        b = c // (NCORES // B)
        t0 = (c % (NCORES // B)) * per_batch
        # padded, transposed h slice  [D, TP]
        hpad = np.zeros((TP, D), f32)
        lo = t0 - PADL
        hi = t0 + T + PADL
        slo, shi = max(lo, 0), min(hi, L)
        hpad[slo - lo:shi - lo] = h_L[b, slo:shi]
        hTa = np.ascontiguousarray(hpad.T).astype(BF16)          # [D, TP]
        hTa = np.ascontiguousarray(hTa.reshape(NKC, 128, TP))

        # validity per (token, offset-order) -> [128, NTC*NOFF]
        tok = t0 + np.arange(T)
        nbr = tok[:, None] + offs[None, :]
        inb = (nbr >= 0) & (nbr < L)
        nbrc = np.clip(nbr, 0, L - 1)
        is_m = (mask[b] == 1)
        val = (inb & is_m[tok][:, None] & (~is_m[nbrc])).astype(f32)
        valT = np.ascontiguousarray(val.T)            # [NOFF, T]
        in_maps.append({
            "hT": hTa,
            "w1a": w1a, "w1b": w1b, "w2": w2, "wr": wr,
            "validT": valT,
            "ones10": np.ones((NOFF, 1), BF16),
            "br_bc": br_bc, "b1s": b1s, "b2s": b2s,
            "ident": identm,
        })
        outs_meta.append((b, t0))
    return in_maps, outs_meta


def kernel(**inputs):
    assert int(inputs["range_r"]) == R
    if "nc" not in _CACHE:
        _CACHE["nc"] = _build_graph()
    nc = _CACHE["nc"]
    in_maps, outs_meta = _prep_shards(
        inputs["h_L"], inputs["mask_flags"], inputs["Wr"], inputs["br"],
        inputs["W1"], inputs["b1"], inputs["W2"], inputs["b2"])
    res = run_bass_kernel_spmd(nc, in_maps, core_ids=list(range(NCORES)))
    out = np.zeros((B, L, D), np.float32)
    for c, (b, t0) in enumerate(outs_meta):
        out[b, t0:t0 + T] = res.results[c]["out"]
    return out


# revision 37
# speedup vs baseline: 1.0145x; 1.0145x over previous
"""Trainium2 Bass kernel for nn_AMIPRouterInference (gnn_message_passing).

Strategy
--------
Algebraic restructure of the reference (~515 GFLOP -> ~52 GFLOP):
  * cond @ W1 splits into h_anc @ W1a + h_ctr @ W1b, each computed once per
    token (not once per window pair):  u = h @ W1b, v = h @ W1a.
  * The attention combine over the +-r window commutes with the W2 matmul:
    g = sum_n cw_n * gelu(v[l+off_n] + u[l]);  delta = (w * g) @ W2 + w @ b2.

Sharding: pure data-parallel over the B*L = 4096 tokens -> 512 tokens/core on
8 cores; the +-5 halo is baked into each core's input shard on the host, so no
collectives are needed.

Per-core layout: features-on-partitions (u/v as 16 chunks of [128, tokens]) so
window shifts along tokens are free-axis SBUF slices.  Even/odd phase copies of
v keep the bf16 DVE 2x alignment for shifted adds.

Engine/queue discipline (all queues are in-order!):
  * PE queue: gram/router MMs -> u/v MMs fc0.. -> cw/w transposes -> more u/v
    MMs -> delta MMs.  Nothing in PE's queue ever waits on the softmax chain
    before matmul work is exhausted.
  * Sync DMA queue: h + small consts -> per-fc W1 streams -> W2 (stage E only)
    -> output stores.  The broadcast DMAs live on the GPSIMD queue, which has
    nothing else to do until the first combine (which needs them anyway).
  * DVE: batched window adds (overlapping-window APs), combine tree; ACT: all
    PSUM evacuation + gelu; GPSIMD: a slice of the cw multiply.
"""

import sys

for _p in ("/opt/trn_rl_repo", "/root/.axon_site/_ro/trn_rl_repo"):
    if _p not in sys.path:
        sys.path.append(_p)

import numpy as np
import ml_dtypes

import bass_rust
import concourse.bacc as bacc
import concourse.mybir as mybir
import concourse.tile as tile
from concourse.bass_utils import run_bass_kernel_spmd

BF16 = ml_dtypes.bfloat16

# Problem constants (hardcoded per spec).
B, L, D = 2, 2048, 1024
K, D4, R = 8, 256, 5
NCORES = 8
T = (B * L) // NCORES          # tokens per core = 512
PADL = 16                      # left pad of the per-core token window
TP = T + 2 * PADL              # padded width = 544
NOFF = 2 * R                   # 10 window offsets
F = K * D4                     # 2048 fused expert features
NFC = F // 128                 # 16 feature chunks
NKC = D // 128                 # 8 contraction chunks
NTC = T // 128                 # 4 token tiles per core

# Offset processing order: even offsets first (read from v_even), then odd
# (read from v_odd, which holds v shifted left by one token).  Within each
# phase the SBUF slice starts are even element indices -> 4-byte aligned.
OFF_ORDER = [-4, -2, 2, 4, -5, -3, -1, 1, 3, 5]

GPS_SPLIT = 3584               # cw-mult columns on GPSIMD; rest on DVE

_CACHE = {}


def _build_graph():
    fp32 = mybir.dt.float32
    bf16 = mybir.dt.bfloat16

    nc = bacc.Bacc("TRN2", target_bir_lowering=False, debug=False,
                   num_devices=NCORES)

    # ---- DRAM parameters (per-core shards; same shapes on every core) ----
    hT = nc.dram_tensor("hT", [NKC, 128, TP], bf16, kind="ExternalInput")
    w1a = nc.dram_tensor("w1a", [NFC, 128, D], bf16, kind="ExternalInput")
    w1b = nc.dram_tensor("w1b", [NFC, 128, D], bf16, kind="ExternalInput")
    w2 = nc.dram_tensor("w2", [NFC, 128, D], bf16, kind="ExternalInput")
    wr = nc.dram_tensor("wr", [NKC, 128, K], bf16, kind="ExternalInput")
    # validity transposed: [NOFF, T]
    validT = nc.dram_tensor("validT", [NOFF, T], fp32, kind="ExternalInput")
    ones10 = nc.dram_tensor("ones10", [NOFF, 1], bf16, kind="ExternalInput")
    br_bc = nc.dram_tensor("br_bc", [128, NTC * K], fp32, kind="ExternalInput")
    b1s = nc.dram_tensor("b1s", [128, NFC], fp32, kind="ExternalInput")
    b2s = nc.dram_tensor("b2s", [K, D], bf16, kind="ExternalInput")
    ident = nc.dram_tensor("ident", [128, 128], fp32, kind="ExternalInput")
    out = nc.dram_tensor("out", [T, D], fp32, kind="ExternalOutput")

    AF = mybir.ActivationFunctionType
    OP = mybir.AluOpType

    def bc_ap(tile_, inner_rep, ncols):
        """[128, ncols] tile viewed as [128, ncols, inner_rep] via a step-0
        innermost dim (per-partition broadcast along the replicated axis)."""
        return bass_rust.AP(
            tensor=tile_[:].tensor, offset=0,
            ap=[[ncols, 128], [1, ncols], [0, inner_rep]])

    with tile.TileContext(nc) as tc:
        with (
            tc.tile_pool(name="const", bufs=1) as cpool,
            tc.tile_pool(name="hpool", bufs=1) as hpool,
            tc.tile_pool(name="w2pool", bufs=1) as w2pool,
            tc.tile_pool(name="w1pool", bufs=2) as w1pool,
            tc.tile_pool(name="small", bufs=2) as spool,
            tc.tile_pool(name="persist", bufs=1) as ppool,
            tc.tile_pool(name="uv", bufs=2) as uvpool,
            tc.tile_pool(name="big", bufs=3) as bigpool,
            tc.tile_pool(name="qbuf", bufs=1) as q1pool,
            tc.tile_pool(name="tbuf", bufs=1) as qpool,
            tc.tile_pool(name="ppart", bufs=1) as partpool,
            tc.tile_pool(name="gout", bufs=1) as gpool,
            tc.tile_pool(name="opool", bufs=2) as opool,
            tc.tile_pool(name="dram", bufs=1, space="DRAM") as dpool,
            tc.tile_pool(name="ps_big", bufs=4, space="PSUM") as psb,
            tc.tile_pool(name="ps_small", bufs=4, space="PSUM") as pss,
            # NOTE: all psb tiles share tag "m" (4 banks), all pss tiles share
            # tag "s" (4 banks) -- PSUM has only 8 banks total.
        ):
            # ---------------- constant / input loads ----------------
            h_sb = []
            for kc in range(NKC):
                t = hpool.tile([128, TP], bf16, tag=f"h{kc}")
                nc.sync.dma_start(t[:], hT[kc])
                h_sb.append(t)
            wr_sb = []
            for kc in range(NKC):
                t = cpool.tile([128, K], bf16, tag=f"wr{kc}")
                nc.sync.dma_start(t[:], wr[kc])
                wr_sb.append(t)
            ident_sb = cpool.tile([128, 128], fp32, tag="ident")
            nc.sync.dma_start(ident_sb[:], ident[:])
            br_sb = cpool.tile([128, NTC * K], fp32, tag="br")
            nc.sync.dma_start(br_sb[:], br_bc[:])
            b1_sb = cpool.tile([128, NFC], fp32, tag="b1")
            nc.sync.dma_start(b1_sb[:], b1s[:])
            b2_sb = cpool.tile([K, D], bf16, tag="b2")
            nc.sync.dma_start(b2_sb[:], b2s[:])
            validT_sb = cpool.tile([NOFF, T], fp32, tag="validT")
            nc.sync.dma_start(validT_sb[:], validT[:])
            ones10_sb = cpool.tile([NOFF, 1], bf16, tag="ones10")
            nc.sync.dma_start(ones10_sb[:], ones10[:])

            # persistent transposed score & router weights (bf16)
            cwT_bf = ppool.tile([NOFF, T], bf16, tag="cwT")
            wT_bf = ppool.tile([K, T], bf16, tag="wT")
            cw_bc = gpool.tile([128, NOFF * 512], bf16, tag="cw_bc")
            w_bc_all = gpool.tile([128, K * 512], bf16, tag="w_bc_all")

            # ------------- stage A/B/C: scores, cw, router w -------------
            # Phase A: all gram/router matmuls first (PE never blocks).
            gram_ps, logit_ps = [], []
            for tci in range(NTC):
                c0 = PADL + tci * 128
                g_ps = psb.tile([128, 512], fp32, tag="m")
                lg_ps = pss.tile([128, K], fp32, tag="s")
                for kc in range(NKC):
                    st = (kc == 0)
                    sp = (kc == NKC - 1)
                    nc.tensor.matmul(g_ps[:, :138],
                                     h_sb[kc][:, c0:c0 + 128],
                                     h_sb[kc][:, c0 - 5:c0 + 133],
                                     start=st, stop=sp)
                    nc.tensor.matmul(lg_ps[:],
                                     h_sb[kc][:, c0:c0 + 128],
                                     wr_sb[kc][:],
                                     start=st, stop=sp)
                gram_ps.append(g_ps)
                logit_ps.append(lg_ps)

            # Phase B1: diagonal extraction, transpose scores to [NOFF, T],
            # then broadcast the UNNORMALIZED ev = exp(s)*valid right away.
            # The softmax 1/den lands at the very end as a per-token (=per-
            # partition) scale on stage E's output.  No max-shift is needed:
            # scores are O(1) so exp() cannot over/underflow.
            s_all = spool.tile([128, NTC * NOFF], fp32, tag="s_all")
            junk = spool.tile([128, 128], fp32, tag="junk")
            for tci in range(NTC):
                g_ps = gram_ps[tci]
                for n, off in enumerate(OFF_ORDER):
                    nc.vector.affine_mul_reduce(
                        junk[:], s_all[:, tci * NOFF + n:tci * NOFF + n + 1],
                        g_ps[:, off + 5:off + 5 + 128], ident_sb[:],
                        1.0 / 32.0, 0.0)
            # router logits evac first: frees the "s" PSUM banks that the
            # score transposes and stage D's vb_ps need.
            lg_all = spool.tile([128, NTC * K], fp32, tag="lg_all")
            for tci in range(NTC):
                nc.scalar.copy(lg_all[:, tci * K:(tci + 1) * K],
                               logit_ps[tci][:])
            sT = ppool.tile([NOFF, T], fp32, tag="sT")
            for tci in range(NTC):
                sT_ps = pss.tile([NOFF, 128], fp32, tag="s")
                nc.tensor.transpose(sT_ps[:],
                                    s_all[:, tci * NOFF:(tci + 1) * NOFF],
                                    ident_sb[:])
                nc.scalar.copy(sT[:, tci * 128:(tci + 1) * 128], sT_ps[:])
            evT = ppool.tile([NOFF, T], fp32, tag="evT")
            nc.scalar.activation(evT[:], sT[:], AF.Exp)
            nc.vector.tensor_mul(cwT_bf[:], evT[:], validT_sb[:])
            cw_dram = dpool.tile([1, NOFF * T], bf16, tag="cw_dram")
            nc.scalar.dma_start(cw_dram[:], cwT_bf[:])
            nc.scalar.dma_start(cw_bc[:],
                                cw_dram[:].partition_broadcast(128))

            rdenT = ppool.tile([128, NTC], fp32, tag="rdenT")
            fT = ppool.tile([128, NTC], fp32, tag="fT")
            wplT_bf = ppool.tile([K, T], bf16, tag="wplT")
            w_pl = ppool.tile([128, NTC * K], fp32, tag="w_pl")

            def phase_b2a():
                """Router softmax + w broadcast (needed by the first
                w-scale)."""
                nc.vector.tensor_add(lg_all[:], lg_all[:], br_sb[:])
                wmx = spool.tile([128, NTC], fp32, tag="wmx")
                lg3 = bass_rust.AP(tensor=lg_all[:].tensor, offset=0,
                                   ap=[[NTC * K, 128], [K, NTC], [1, K]])
                nc.vector.reduce_max(wmx[:], lg3, mybir.AxisListType.X)
                we = spool.tile([128, NTC * K], fp32, tag="we")
                nc.vector.tensor_sub(we[:], lg_all[:], bc_ap(wmx, K, NTC))
                nc.scalar.activation(we[:], we[:], AF.Exp)
                wsum = spool.tile([128, NTC], fp32, tag="wsum")
                we3 = bass_rust.AP(tensor=we[:].tensor, offset=0,
                                   ap=[[NTC * K, 128], [K, NTC], [1, K]])
                nc.vector.reduce_sum(wsum[:], we3, mybir.AxisListType.X)
                rws = spool.tile([128, NTC], fp32, tag="rws")
                nc.vector.reciprocal(rws[:], wsum[:])
                nc.vector.tensor_mul(w_pl[:], we[:], bc_ap(rws, K, NTC))
                for tci in range(NTC):
                    wpT_ps = pss.tile([K, 128], fp32, tag="s")
                    nc.tensor.transpose(wpT_ps[:],
                                        w_pl[:, tci * K:(tci + 1) * K],
                                        ident_sb[:])
                    nc.scalar.copy(wplT_bf[:, tci * 128:(tci + 1) * 128],
                                   wpT_ps[:])
                w_dram = dpool.tile([1, K * T], bf16, tag="w_dram")
                nc.scalar.dma_start(w_dram[:], wplT_bf[:])
                nc.scalar.dma_start(w_bc_all[:],
                                    w_dram[:].partition_broadcast(128))

            def phase_b2b():
                """Denominators + per-token scale columns + b2-path weights
                (needed only from stage E1 onward)."""
                den_ps = pss.tile([1, T], fp32, tag="s")
                nc.tensor.matmul(den_ps[:], ones10_sb[:], cwT_bf[:],
                                 start=True, stop=True)
                den = ppool.tile([1, T], fp32, tag="den")
                nc.scalar.copy(den[:], den_ps[:])       # raw sum_n ev
                dene = ppool.tile([1, T], fp32, tag="dene")
                nc.vector.tensor_scalar_add(dene[:], den[:], 1e-30)
                rden = ppool.tile([1, T], fp32, tag="rden")
                nc.vector.reciprocal(rden[:], dene[:])
                for tci in range(NTC):
                    nc.scalar.dma_start(rdenT[:, tci:tci + 1],
                                        rden[:, tci * 128:(tci + 1) * 128])
                    nc.scalar.dma_start(fT[:, tci:tci + 1],
                                        den[:, tci * 128:(tci + 1) * 128])
                # b2-path weights: w * raw_den (so the final 1/den scale on
                # stage E's output reproduces w * sum_cw exactly)
                weff3 = spool.tile([128, NTC * K], fp32, tag="weff3")
                for tci in range(NTC):
                    nc.vector.tensor_scalar_mul(
                        weff3[:, tci * K:(tci + 1) * K],
                        w_pl[:, tci * K:(tci + 1) * K], fT[:, tci:tci + 1])
                for tci in range(NTC):
                    weT_ps = pss.tile([K, 128], fp32, tag="s")
                    nc.tensor.transpose(weT_ps[:],
                                        weff3[:, tci * K:(tci + 1) * K],
                                        ident_sb[:])
                    nc.scalar.copy(wT_bf[:, tci * 128:(tci + 1) * 128],
                                   weT_ps[:])

            # ------------- stage D: u/v matmuls + gelu combine -------------
            g_sb = [None] * NFC
            tmp_sb = [None] * NFC

            def stage_d_mm(fc):
                w1b_t = w1pool.tile([128, D], bf16, tag="w1b")
                nc.sync.dma_start(w1b_t[:], w1b[fc])
                w1a_t = w1pool.tile([128, D], bf16, tag="w1a")
                nc.sync.dma_start(w1a_t[:], w1a[fc])

                u_ps = psb.tile([128, 512], fp32, tag="m")
                va_ps = psb.tile([128, 512], fp32, tag="m")
                vb_ps = pss.tile([128, 48], fp32, tag="s")
                for kc in range(NKC):
                    st = (kc == 0)
                    sp = (kc == NKC - 1)
                    lhs_b = w1b_t[:, kc * 128:(kc + 1) * 128]
                    lhs_a = w1a_t[:, kc * 128:(kc + 1) * 128]
                    nc.tensor.matmul(u_ps[:], lhs_b,
                                     h_sb[kc][:, PADL:PADL + 512],
                                     start=st, stop=sp)
                    nc.tensor.matmul(va_ps[:], lhs_a,
                                     h_sb[kc][:, 0:512],
                                     start=st, stop=sp)
                    nc.tensor.matmul(vb_ps[:], lhs_a,
                                     h_sb[kc][:, 496:544],
                                     start=st, stop=sp)

                u_sb = uvpool.tile([128, 512], bf16, tag="u")
                nc.scalar.copy(u_sb[:], u_ps[:])
                v_ev = uvpool.tile([128, TP], bf16, tag="v_ev")
                nc.scalar.copy(v_ev[:, 0:512], va_ps[:])
                nc.scalar.copy(v_ev[:, 512:544], vb_ps[:, 16:48])
                # odd phase built straight from PSUM (keeps DMA out of the
                # critical chain)
                v_od = uvpool.tile([128, TP], bf16, tag="v_od")
                nc.scalar.copy(v_od[:, 0:511], va_ps[:, 1:512])
                nc.scalar.copy(v_od[:, 511:543], vb_ps[:, 16:48])

                tmp = bigpool.tile([128, NOFF * 512], bf16, tag="tmp")
                # 10 contiguous shifted adds -- contiguous 1D free APs keep
                # the DVE in its bf16 2x perf mode (overlapping multi-dim
                # APs drop to 1x).
                for n, off in enumerate(OFF_ORDER):
                    if off % 2 == 0:
                        vsrc = v_ev[:, PADL + off:PADL + off + 512]
                    else:
                        vsrc = v_od[:, PADL - 1 + off:PADL - 1 + off + 512]
                    nc.vector.tensor_add(
                        tmp[:, n * 512:(n + 1) * 512], vsrc, u_sb[:])
                nc.scalar.activation(tmp[:], tmp[:], AF.Gelu,
                                     bias=b1_sb[:, fc:fc + 1])
                tmp_sb[fc] = tmp

            def stage_d_combine(fc):
                tmp = tmp_sb[fc]
                q = q1pool.tile([128, NOFF * 512], bf16, tag="q")
                # NOTE: GPSIMD shares SBUF ports with DVE -- offloading
                # elementwise work there just steals DVE bandwidth, so the
                # whole multiply stays on DVE.
                nc.vector.tensor_mul(q[:], tmp[:], cw_bc[:])

                # pairwise tree-sum of the 10 weighted slices, then w-scale
                t1 = qpool.tile([128, 2560], bf16, tag="t1")
                nc.vector.tensor_add(t1[:], q[:, 0:2560], q[:, 2560:5120])
                t2 = qpool.tile([128, 1024], bf16, tag="t2")
                nc.vector.tensor_add(t2[:], t1[:, 0:1024], t1[:, 1024:2048])
                t3 = qpool.tile([128, 512], bf16, tag="t3")
                nc.vector.tensor_add(t3[:], t2[:, 0:512], t2[:, 512:1024])
                t4 = qpool.tile([128, 512], bf16, tag="t4")
                nc.vector.tensor_add(t4[:], t3[:], t1[:, 2048:2560])
                g_t = gpool.tile([128, 512], bf16, tag=f"g{fc}")
                nc.vector.tensor_mul(
                    g_t[:], t4[:],
                    w_bc_all[:, (fc // 2) * 512:(fc // 2) * 512 + 512])
                g_sb[fc] = g_t

            w2_sb = [None] * NFC

            def load_w2(j):
                t = w2pool.tile([128, D], bf16, tag=f"w2_{j}")
                nc.sync.dma_start(t[:], w2[j])
                w2_sb[j] = t

            d_part = [None] * 8

            def stage_e1(blk):
                """Partial delta over fc 0..7 -- interleaved mid stage-D."""
                tci, dh = blk // 2, blk % 2
                d_ps = pss.tile([128, 512], fp32, tag="s")
                for fc in range(8):
                    nc.tensor.matmul(
                        d_ps[:],
                        g_sb[fc][:, tci * 128:(tci + 1) * 128],
                        w2_sb[fc][:, dh * 512:(dh + 1) * 512],
                        start=(fc == 0), stop=(fc == 7))
                p_t = partpool.tile([128, 512], bf16, tag=f"p{blk}")
                nc.scalar.mul(p_t[:], d_ps[:], rdenT[:, tci:tci + 1])
                d_part[blk] = p_t

            def stage_e2(blk):
                tci, dh = blk // 2, blk % 2
                d_ps = pss.tile([128, 512], fp32, tag="s")
                for fc in range(8, NFC):
                    nc.tensor.matmul(
                        d_ps[:],
                        g_sb[fc][:, tci * 128:(tci + 1) * 128],
                        w2_sb[fc][:, dh * 512:(dh + 1) * 512],
                        start=(fc == 8), stop=False)
                nc.tensor.matmul(
                    d_ps[:],
                    wT_bf[:, tci * 128:(tci + 1) * 128],
                    b2_sb[:, dh * 512:(dh + 1) * 512],
                    start=False, stop=True)
                o_sb = opool.tile([128, 512], fp32, tag="o")
                nc.vector.scalar_tensor_tensor(
                    o_sb[:], d_ps[:], rdenT[:, tci:tci + 1], d_part[blk][:],
                    op0=OP.mult, op1=OP.add)
                nc.sync.dma_start(
                    out[tci * 128:(tci + 1) * 128,
                        dh * 512:(dh + 1) * 512], o_sb[:])

            # ---- emission schedule ----
            # 2-fc runway hides the softmax->broadcast latency; combines then
            # run at lag 1.  E1 (delta over fc 0..7) interleaves into the
            # back half; the tail is combine(15) + E2 only.
            stage_d_mm(0)
            stage_d_mm(1)
            phase_b2a()
            stage_d_combine(0)
            e1_done = 0
            for fc in range(2, NFC):
                stage_d_mm(fc)
                stage_d_combine(fc - 1)
                if 2 <= fc < 10:
                    load_w2(fc - 2)         # w2 chunks 0..7 for E1
                    load_w2(fc + 6)         # w2 chunks 8..15 for E2
                if fc == 8:
                    phase_b2b()
                if fc >= 11:                # g 0..7 + w2 0..7 ready
                    for _ in range(2):
                        if e1_done < 8:
                            stage_e1(e1_done)
                            e1_done += 1
            stage_d_combine(NFC - 1)
            while e1_done < 8:
                stage_e1(e1_done)
                e1_done += 1
            for blk in range(8):
                stage_e2(blk)

    nc.compile()
    return nc


def _prep_shards(h_L, mask_flags, Wr, br, W1, b1, W2, b2):
    """Host-side shard construction (numpy only; cheap vs device work)."""
    f32 = np.float32
    h_L = np.asarray(h_L, f32)
    mask = np.asarray(mask_flags)
    Wr = np.asarray(Wr, f32)
    W1 = np.asarray(W1, f32)
    W2 = np.asarray(W2, f32)
    br = np.asarray(br, f32)
    b1 = np.asarray(b1, f32)
    b2 = np.asarray(b2, f32)

    # shared (replicated) weight blocks
    w1a = np.ascontiguousarray(
        W1[:, :D, :].transpose(1, 0, 2).reshape(D, F)
        .reshape(NKC, 128, NFC, 128).transpose(2, 1, 0, 3)
        .reshape(NFC, 128, D)).astype(BF16)
    w1b = np.ascontiguousarray(
        W1[:, D:, :].transpose(1, 0, 2).reshape(D, F)
        .reshape(NKC, 128, NFC, 128).transpose(2, 1, 0, 3)
        .reshape(NFC, 128, D)).astype(BF16)
    w2 = np.ascontiguousarray(W2.reshape(F, D).reshape(NFC, 128, D)).astype(BF16)
    wr = np.ascontiguousarray(Wr.reshape(NKC, 128, K)).astype(BF16)
    br_bc = np.tile(np.broadcast_to(br[None, :], (128, K)), (1, NTC)).astype(f32)
    b1s = np.ascontiguousarray(b1.reshape(F).reshape(NFC, 128).T)
    b2s = b2.astype(BF16)
    identm = np.eye(128, dtype=f32)

    offs = np.array(OFF_ORDER, np.int64)
    in_maps = []
    outs_meta = []
    per_batch = L // (NCORES // B)          # 512 tokens, 4 shards per batch
    for c in range(NCORES):
        b = c // (NCORES // B)
        t0 = (c % (NCORES // B)) * per_batch
        # padded, transposed h slice  [D, TP]
        hpad = np.zeros((TP, D), f32)
        lo = t0 - PADL
        hi = t0 + T + PADL
        slo, shi = max(lo, 0), min(hi, L)
        hpad[slo - lo:shi - lo] = h_L[b, slo:shi]
        hTa = np.ascontiguousarray(hpad.T).astype(BF16)          # [D, TP]
        hTa = np.ascontiguousarray(hTa.reshape(NKC, 128, TP))

        # validity per (token, offset-order) -> [128, NTC*NOFF]
        tok = t0 + np.arange(T)
        nbr = tok[:, None] + offs[None, :]
        inb = (nbr >= 0) & (nbr < L)
        nbrc = np.clip(nbr, 0, L - 1)
        is_m = (mask[b] == 1)
        val = (inb & is_m[tok][:, None] & (~is_m[nbrc])).astype(f32)
        valT = np.ascontiguousarray(val.T)            # [NOFF, T]
        in_maps.append({
            "hT": hTa,
            "w1a": w1a, "w1b": w1b, "w2": w2, "wr": wr,
            "validT": valT,
            "ones10": np.ones((NOFF, 1), BF16),
            "br_bc": br_bc, "b1s": b1s, "b2s": b2s,
            "ident": identm,
        })
        outs_meta.append((b, t0))
    return in_maps, outs_meta


def kernel(**inputs):
    assert int(inputs["range_r"]) == R
    if "nc" not in _CACHE:
        _CACHE["nc"] = _build_graph()
    nc = _CACHE["nc"]
    in_maps, outs_meta = _prep_shards(
        inputs["h_L"], inputs["mask_flags"], inputs["Wr"], inputs["br"],
        inputs["W1"], inputs["b1"], inputs["W2"], inputs["b2"])
    res = run_bass_kernel_spmd(nc, in_maps, core_ids=list(range(NCORES)))
    out = np.zeros((B, L, D), np.float32)
    for c, (b, t0) in enumerate(outs_meta):
        out[b, t0:t0 + T] = res.results[c]["out"]
    return out


# revision 38
# speedup vs baseline: 1.1878x; 1.1708x over previous
"""Trainium2 Bass kernel for nn_AMIPRouterInference (gnn_message_passing).

Strategy
--------
Algebraic restructure of the reference (~515 GFLOP -> ~52 GFLOP):
  * cond @ W1 splits into h_anc @ W1a + h_ctr @ W1b, each computed once per
    token (not once per window pair):  u = h @ W1b, v = h @ W1a.
  * The attention combine over the +-r window commutes with the W2 matmul:
    g = sum_n cw_n * gelu(v[l+off_n] + u[l]);  delta = (w * g) @ W2 + w @ b2.

Sharding: pure data-parallel over the B*L = 4096 tokens -> 512 tokens/core on
8 cores; the +-5 halo is baked into each core's input shard on the host, so no
collectives are needed.

Per-core layout: features-on-partitions (u/v as 16 chunks of [128, tokens]) so
window shifts along tokens are free-axis SBUF slices.  Even/odd phase copies of
v keep the bf16 DVE 2x alignment for shifted adds.

Engine/queue discipline (all queues are in-order!):
  * PE queue: gram/router MMs -> u/v MMs fc0.. -> cw/w transposes -> more u/v
    MMs -> delta MMs.  Nothing in PE's queue ever waits on the softmax chain
    before matmul work is exhausted.
  * Sync DMA queue: h + small consts -> per-fc W1 streams -> W2 (stage E only)
    -> output stores.  The broadcast DMAs live on the GPSIMD queue, which has
    nothing else to do until the first combine (which needs them anyway).
  * DVE: batched window adds (overlapping-window APs), combine tree; ACT: all
    PSUM evacuation + gelu; GPSIMD: a slice of the cw multiply.
"""

import sys

for _p in ("/opt/trn_rl_repo", "/root/.axon_site/_ro/trn_rl_repo"):
    if _p not in sys.path:
        sys.path.append(_p)

import numpy as np
import ml_dtypes

import bass_rust
import concourse.bacc as bacc
import concourse.mybir as mybir
import concourse.tile as tile
from concourse.bass_utils import run_bass_kernel_spmd

BF16 = ml_dtypes.bfloat16

# Problem constants (hardcoded per spec).
B, L, D = 2, 2048, 1024
K, D4, R = 8, 256, 5
NCORES = 8
T = (B * L) // NCORES          # tokens per core = 512
PADL = 16                      # left pad of the per-core token window
TP = T + 2 * PADL              # padded width = 544
NOFF = 2 * R                   # 10 window offsets
F = K * D4                     # 2048 fused expert features
NFC = F // 128                 # 16 feature chunks
NKC = D // 128                 # 8 contraction chunks
NTC = T // 128                 # 4 token tiles per core

# Offset processing order: even offsets first (read from v_even), then odd
# (read from v_odd, which holds v shifted left by one token).  Within each
# phase the SBUF slice starts are even element indices -> 4-byte aligned.
OFF_ORDER = [-4, -2, 2, 4, -5, -3, -1, 1, 3, 5]

GPS_SPLIT = 3584               # cw-mult columns on GPSIMD; rest on DVE

_CACHE = {}


def _build_graph():
    fp32 = mybir.dt.float32
    bf16 = mybir.dt.bfloat16

    nc = bacc.Bacc("TRN2", target_bir_lowering=False, debug=False,
                   num_devices=NCORES)

    # ---- DRAM parameters (per-core shards; same shapes on every core) ----
    hT = nc.dram_tensor("hT", [NKC, 128, TP], bf16, kind="ExternalInput")
    w1a = nc.dram_tensor("w1a", [NFC, 128, D], bf16, kind="ExternalInput")
    w1b = nc.dram_tensor("w1b", [NFC, 128, D], bf16, kind="ExternalInput")
    w2 = nc.dram_tensor("w2", [NFC, 128, D], bf16, kind="ExternalInput")
    wr = nc.dram_tensor("wr", [NKC, 128, K], bf16, kind="ExternalInput")
    # validity transposed: [NOFF, T]
    validT = nc.dram_tensor("validT", [NOFF, T], fp32, kind="ExternalInput")
    ones10 = nc.dram_tensor("ones10", [NOFF, 1], bf16, kind="ExternalInput")
    br_bc = nc.dram_tensor("br_bc", [128, NTC * K], fp32, kind="ExternalInput")
    b1s = nc.dram_tensor("b1s", [128, NFC], fp32, kind="ExternalInput")
    b2s = nc.dram_tensor("b2s", [K, D], bf16, kind="ExternalInput")
    ident = nc.dram_tensor("ident", [128, 128], fp32, kind="ExternalInput")
    out = nc.dram_tensor("out", [T, D], fp32, kind="ExternalOutput")

    AF = mybir.ActivationFunctionType
    OP = mybir.AluOpType

    def bc_ap(tile_, inner_rep, ncols):
        """[128, ncols] tile viewed as [128, ncols, inner_rep] via a step-0
        innermost dim (per-partition broadcast along the replicated axis)."""
        return bass_rust.AP(
            tensor=tile_[:].tensor, offset=0,
            ap=[[ncols, 128], [1, ncols], [0, inner_rep]])

    with tile.TileContext(nc) as tc:
        with (
            tc.tile_pool(name="const", bufs=1) as cpool,
            tc.tile_pool(name="hpool", bufs=1) as hpool,
            tc.tile_pool(name="w2pool", bufs=1) as w2pool,
            tc.tile_pool(name="w1pool", bufs=2) as w1pool,
            tc.tile_pool(name="small", bufs=2) as spool,
            tc.tile_pool(name="persist", bufs=1) as ppool,
            tc.tile_pool(name="uv", bufs=2) as uvpool,
            tc.tile_pool(name="big", bufs=3) as bigpool,
            tc.tile_pool(name="qbuf", bufs=2) as q1pool,
            tc.tile_pool(name="tbuf", bufs=1) as qpool,
            tc.tile_pool(name="ppart", bufs=1) as partpool,
            tc.tile_pool(name="gout", bufs=1) as gpool,
            tc.tile_pool(name="opool", bufs=2) as opool,
            tc.tile_pool(name="dram", bufs=1, space="DRAM") as dpool,
            tc.tile_pool(name="ps_big", bufs=4, space="PSUM") as psb,
            tc.tile_pool(name="ps_small", bufs=4, space="PSUM") as pss,
            # NOTE: all psb tiles share tag "m" (4 banks), all pss tiles share
            # tag "s" (4 banks) -- PSUM has only 8 banks total.
        ):
            # ---------------- constant / input loads ----------------
            h_sb = []
            for kc in range(NKC):
                t = hpool.tile([128, TP], bf16, tag=f"h{kc}")
                nc.sync.dma_start(t[:], hT[kc])
                h_sb.append(t)
            wr_sb = []
            for kc in range(NKC):
                t = cpool.tile([128, K], bf16, tag=f"wr{kc}")
                nc.sync.dma_start(t[:], wr[kc])
                wr_sb.append(t)
            ident_sb = cpool.tile([128, 128], fp32, tag="ident")
            nc.sync.dma_start(ident_sb[:], ident[:])
            br_sb = cpool.tile([128, NTC * K], fp32, tag="br")
            nc.sync.dma_start(br_sb[:], br_bc[:])
            b1_sb = cpool.tile([128, NFC], fp32, tag="b1")
            nc.sync.dma_start(b1_sb[:], b1s[:])
            b2_sb = cpool.tile([K, D], bf16, tag="b2")
            nc.sync.dma_start(b2_sb[:], b2s[:])
            validT_sb = cpool.tile([NOFF, T], fp32, tag="validT")
            nc.sync.dma_start(validT_sb[:], validT[:])
            ones10_sb = cpool.tile([NOFF, 1], bf16, tag="ones10")
            nc.sync.dma_start(ones10_sb[:], ones10[:])

            # persistent transposed score & router weights (bf16)
            cwT_bf = ppool.tile([NOFF, T], bf16, tag="cwT")
            wT_bf = ppool.tile([K, T], bf16, tag="wT")
            cw_bc = gpool.tile([128, NOFF * 512], bf16, tag="cw_bc")
            w_bc_all = gpool.tile([128, K * 512], bf16, tag="w_bc_all")

            # ------------- stage A/B/C: scores, cw, router w -------------
            # Phase A: all gram/router matmuls first (PE never blocks).
            gram_ps, logit_ps = [], []
            for tci in range(NTC):
                c0 = PADL + tci * 128
                g_ps = psb.tile([128, 512], fp32, tag="m")
                lg_ps = pss.tile([128, K], fp32, tag="s")
                for kc in range(NKC):
                    st = (kc == 0)
                    sp = (kc == NKC - 1)
                    nc.tensor.matmul(g_ps[:, :138],
                                     h_sb[kc][:, c0:c0 + 128],
                                     h_sb[kc][:, c0 - 5:c0 + 133],
                                     start=st, stop=sp)
                    nc.tensor.matmul(lg_ps[:],
                                     h_sb[kc][:, c0:c0 + 128],
                                     wr_sb[kc][:],
                                     start=st, stop=sp)
                gram_ps.append(g_ps)
                logit_ps.append(lg_ps)

            # Phase B1: diagonal extraction, transpose scores to [NOFF, T],
            # then broadcast the UNNORMALIZED ev = exp(s)*valid right away.
            # The softmax 1/den lands at the very end as a per-token (=per-
            # partition) scale on stage E's output.  No max-shift is needed:
            # scores are O(1) so exp() cannot over/underflow.
            s_all = spool.tile([128, NTC * NOFF], fp32, tag="s_all")
            junk = spool.tile([128, 128], fp32, tag="junk")
            for tci in range(NTC):
                g_ps = gram_ps[tci]
                for n, off in enumerate(OFF_ORDER):
                    nc.vector.affine_mul_reduce(
                        junk[:], s_all[:, tci * NOFF + n:tci * NOFF + n + 1],
                        g_ps[:, off + 5:off + 5 + 128], ident_sb[:],
                        1.0 / 32.0, 0.0)
            # router logits evac first: frees the "s" PSUM banks that the
            # score transposes and stage D's vb_ps need.
            lg_all = spool.tile([128, NTC * K], fp32, tag="lg_all")
            for tci in range(NTC):
                nc.scalar.copy(lg_all[:, tci * K:(tci + 1) * K],
                               logit_ps[tci][:])
            sT = ppool.tile([NOFF, T], fp32, tag="sT")
            for tci in range(NTC):
                sT_ps = pss.tile([NOFF, 128], fp32, tag="s")
                nc.tensor.transpose(sT_ps[:],
                                    s_all[:, tci * NOFF:(tci + 1) * NOFF],
                                    ident_sb[:])
                nc.scalar.copy(sT[:, tci * 128:(tci + 1) * 128], sT_ps[:])
            evT = ppool.tile([NOFF, T], fp32, tag="evT")
            nc.scalar.activation(evT[:], sT[:], AF.Exp)
            nc.vector.tensor_mul(cwT_bf[:], evT[:], validT_sb[:])
            cw_dram = dpool.tile([1, NOFF * T], bf16, tag="cw_dram")
            nc.scalar.dma_start(cw_dram[:], cwT_bf[:])
            nc.scalar.dma_start(cw_bc[:],
                                cw_dram[:].partition_broadcast(128))

            rdenT = ppool.tile([128, NTC], fp32, tag="rdenT")
            fT = ppool.tile([128, NTC], fp32, tag="fT")
            wplT_bf = ppool.tile([K, T], bf16, tag="wplT")
            w_pl = ppool.tile([128, NTC * K], fp32, tag="w_pl")

            def phase_b2a():
                """Router softmax + w broadcast (needed by the first
                w-scale)."""
                nc.vector.tensor_add(lg_all[:], lg_all[:], br_sb[:])
                wmx = spool.tile([128, NTC], fp32, tag="wmx")
                lg3 = bass_rust.AP(tensor=lg_all[:].tensor, offset=0,
                                   ap=[[NTC * K, 128], [K, NTC], [1, K]])
                nc.vector.reduce_max(wmx[:], lg3, mybir.AxisListType.X)
                we = spool.tile([128, NTC * K], fp32, tag="we")
                nc.vector.tensor_sub(we[:], lg_all[:], bc_ap(wmx, K, NTC))
                nc.scalar.activation(we[:], we[:], AF.Exp)
                wsum = spool.tile([128, NTC], fp32, tag="wsum")
                we3 = bass_rust.AP(tensor=we[:].tensor, offset=0,
                                   ap=[[NTC * K, 128], [K, NTC], [1, K]])
                nc.vector.reduce_sum(wsum[:], we3, mybir.AxisListType.X)
                rws = spool.tile([128, NTC], fp32, tag="rws")
                nc.vector.reciprocal(rws[:], wsum[:])
                nc.vector.tensor_mul(w_pl[:], we[:], bc_ap(rws, K, NTC))
                for tci in range(NTC):
                    wpT_ps = pss.tile([K, 128], fp32, tag="s")
                    nc.tensor.transpose(wpT_ps[:],
                                        w_pl[:, tci * K:(tci + 1) * K],
                                        ident_sb[:])
                    nc.scalar.copy(wplT_bf[:, tci * 128:(tci + 1) * 128],
                                   wpT_ps[:])
                w_dram = dpool.tile([1, K * T], bf16, tag="w_dram")
                nc.scalar.dma_start(w_dram[:], wplT_bf[:])
                nc.scalar.dma_start(w_bc_all[:],
                                    w_dram[:].partition_broadcast(128))

            def phase_b2b():
                """Denominators + per-token scale columns + b2-path weights
                (needed only from stage E1 onward)."""
                den_ps = pss.tile([1, T], fp32, tag="s")
                nc.tensor.matmul(den_ps[:], ones10_sb[:], cwT_bf[:],
                                 start=True, stop=True)
                den = ppool.tile([1, T], fp32, tag="den")
                nc.scalar.copy(den[:], den_ps[:])       # raw sum_n ev
                dene = ppool.tile([1, T], fp32, tag="dene")
                nc.vector.tensor_scalar_add(dene[:], den[:], 1e-30)
                rden = ppool.tile([1, T], fp32, tag="rden")
                nc.vector.reciprocal(rden[:], dene[:])
                for tci in range(NTC):
                    nc.scalar.dma_start(rdenT[:, tci:tci + 1],
                                        rden[:, tci * 128:(tci + 1) * 128])
                    nc.scalar.dma_start(fT[:, tci:tci + 1],
                                        den[:, tci * 128:(tci + 1) * 128])
                # b2-path weights: w * raw_den (so the final 1/den scale on
                # stage E's output reproduces w * sum_cw exactly)
                weff3 = spool.tile([128, NTC * K], fp32, tag="weff3")
                for tci in range(NTC):
                    nc.vector.tensor_scalar_mul(
                        weff3[:, tci * K:(tci + 1) * K],
                        w_pl[:, tci * K:(tci + 1) * K], fT[:, tci:tci + 1])
                for tci in range(NTC):
                    weT_ps = pss.tile([K, 128], fp32, tag="s")
                    nc.tensor.transpose(weT_ps[:],
                                        weff3[:, tci * K:(tci + 1) * K],
                                        ident_sb[:])
                    nc.scalar.copy(wT_bf[:, tci * 128:(tci + 1) * 128],
                                   weT_ps[:])

            # ------------- stage D: u/v matmuls + gelu combine -------------
            g_sb = [None] * NFC
            tmp_sb = [None] * NFC

            def stage_d_mm(fc):
                w1b_t = w1pool.tile([128, D], bf16, tag="w1b")
                nc.sync.dma_start(w1b_t[:], w1b[fc])
                w1a_t = w1pool.tile([128, D], bf16, tag="w1a")
                nc.sync.dma_start(w1a_t[:], w1a[fc])

                u_ps = psb.tile([128, 512], fp32, tag="m")
                va_ps = psb.tile([128, 512], fp32, tag="m")
                vb_ps = pss.tile([128, 48], fp32, tag="s")
                for kc in range(NKC):
                    st = (kc == 0)
                    sp = (kc == NKC - 1)
                    lhs_b = w1b_t[:, kc * 128:(kc + 1) * 128]
                    lhs_a = w1a_t[:, kc * 128:(kc + 1) * 128]
                    nc.tensor.matmul(u_ps[:], lhs_b,
                                     h_sb[kc][:, PADL:PADL + 512],
                                     start=st, stop=sp)
                    nc.tensor.matmul(va_ps[:], lhs_a,
                                     h_sb[kc][:, 0:512],
                                     start=st, stop=sp)
                    nc.tensor.matmul(vb_ps[:], lhs_a,
                                     h_sb[kc][:, 496:544],
                                     start=st, stop=sp)

                u_sb = uvpool.tile([128, 512], bf16, tag="u")
                nc.scalar.copy(u_sb[:], u_ps[:])
                v_ev = uvpool.tile([128, TP], bf16, tag="v_ev")
                nc.scalar.copy(v_ev[:, 0:512], va_ps[:])
                nc.scalar.copy(v_ev[:, 512:544], vb_ps[:, 16:48])
                # odd phase built straight from PSUM (keeps DMA out of the
                # critical chain)
                v_od = uvpool.tile([128, TP], bf16, tag="v_od")
                nc.scalar.copy(v_od[:, 0:511], va_ps[:, 1:512])
                nc.scalar.copy(v_od[:, 511:543], vb_ps[:, 16:48])

                tmp = bigpool.tile([128, NOFF * 512], bf16, tag="tmp")
                # 10 contiguous shifted adds -- contiguous 1D free APs keep
                # the DVE in its bf16 2x perf mode (overlapping multi-dim
                # APs drop to 1x).
                for n, off in enumerate(OFF_ORDER):
                    if off % 2 == 0:
                        vsrc = v_ev[:, PADL + off:PADL + off + 512]
                    else:
                        vsrc = v_od[:, PADL - 1 + off:PADL - 1 + off + 512]
                    nc.vector.tensor_add(
                        tmp[:, n * 512:(n + 1) * 512], vsrc, u_sb[:])
                nc.scalar.activation(tmp[:], tmp[:], AF.Gelu,
                                     bias=b1_sb[:, fc:fc + 1])
                tmp_sb[fc] = tmp

            def stage_d_combine(fc):
                tmp = tmp_sb[fc]
                q = q1pool.tile([128, NOFF * 512], bf16, tag="q")
                # NOTE: GPSIMD shares SBUF ports with DVE -- offloading
                # elementwise work there just steals DVE bandwidth, so the
                # whole multiply stays on DVE.
                nc.vector.tensor_mul(q[:], tmp[:], cw_bc[:])

                # pairwise tree-sum of the 10 weighted slices, then w-scale
                t1 = qpool.tile([128, 2560], bf16, tag="t1")
                nc.vector.tensor_add(t1[:], q[:, 0:2560], q[:, 2560:5120])
                t2 = qpool.tile([128, 1024], bf16, tag="t2")
                nc.vector.tensor_add(t2[:], t1[:, 0:1024], t1[:, 1024:2048])
                t3 = qpool.tile([128, 512], bf16, tag="t3")
                nc.vector.tensor_add(t3[:], t2[:, 0:512], t2[:, 512:1024])
                t4 = qpool.tile([128, 512], bf16, tag="t4")
                nc.vector.tensor_add(t4[:], t3[:], t1[:, 2048:2560])
                g_t = gpool.tile([128, 512], bf16, tag=f"g{fc}")
                nc.vector.tensor_mul(
                    g_t[:], t4[:],
                    w_bc_all[:, (fc // 2) * 512:(fc // 2) * 512 + 512])
                g_sb[fc] = g_t

            w2_sb = [None] * NFC

            def load_w2(j):
                t = w2pool.tile([128, D], bf16, tag=f"w2_{j}")
                nc.sync.dma_start(t[:], w2[j])
                w2_sb[j] = t

            d_part = [None] * 8

            def stage_e1(blk):
                """Partial delta over fc 0..7 -- interleaved mid stage-D."""
                tci, dh = blk // 2, blk % 2
                d_ps = pss.tile([128, 512], fp32, tag="s")
                for fc in range(8):
                    nc.tensor.matmul(
                        d_ps[:],
                        g_sb[fc][:, tci * 128:(tci + 1) * 128],
                        w2_sb[fc][:, dh * 512:(dh + 1) * 512],
                        start=(fc == 0), stop=(fc == 7))
                p_t = partpool.tile([128, 512], bf16, tag=f"p{blk}")
                nc.scalar.mul(p_t[:], d_ps[:], rdenT[:, tci:tci + 1])
                d_part[blk] = p_t

            def stage_e2(blk):
                tci, dh = blk // 2, blk % 2
                d_ps = pss.tile([128, 512], fp32, tag="s")
                for fc in range(8, NFC):
                    nc.tensor.matmul(
                        d_ps[:],
                        g_sb[fc][:, tci * 128:(tci + 1) * 128],
                        w2_sb[fc][:, dh * 512:(dh + 1) * 512],
                        start=(fc == 8), stop=False)
                nc.tensor.matmul(
                    d_ps[:],
                    wT_bf[:, tci * 128:(tci + 1) * 128],
                    b2_sb[:, dh * 512:(dh + 1) * 512],
                    start=False, stop=True)
                o_sb = opool.tile([128, 512], fp32, tag="o")
                nc.vector.scalar_tensor_tensor(
                    o_sb[:], d_ps[:], rdenT[:, tci:tci + 1], d_part[blk][:],
                    op0=OP.mult, op1=OP.add)
                nc.sync.dma_start(
                    out[tci * 128:(tci + 1) * 128,
                        dh * 512:(dh + 1) * 512], o_sb[:])

            # ---- emission schedule ----
            # 2-fc runway hides the softmax->broadcast latency; combines then
            # run at lag 1.  E1 (delta over fc 0..7) interleaves into the
            # back half; the tail is combine(15) + E2 only.
            stage_d_mm(0)
            stage_d_mm(1)
            phase_b2a()
            stage_d_combine(0)
            e1_done = 0
            for fc in range(2, NFC):
                stage_d_mm(fc)
                stage_d_combine(fc - 1)
                if 2 <= fc < 10:
                    load_w2(fc - 2)         # w2 chunks 0..7 for E1
                    load_w2(fc + 6)         # w2 chunks 8..15 for E2
                if fc == 8:
                    phase_b2b()
                if fc >= 11:                # g 0..7 + w2 0..7 ready
                    for _ in range(2):
                        if e1_done < 8:
                            stage_e1(e1_done)
                            e1_done += 1
            stage_d_combine(NFC - 1)
            while e1_done < 8:
                stage_e1(e1_done)
                e1_done += 1
            for blk in range(8):
                stage_e2(blk)

    nc.compile()
    return nc


def _prep_shards(h_L, mask_flags, Wr, br, W1, b1, W2, b2):
    """Host-side shard construction (numpy only; cheap vs device work)."""
    f32 = np.float32
    h_L = np.asarray(h_L, f32)
    mask = np.asarray(mask_flags)
    Wr = np.asarray(Wr, f32)
    W1 = np.asarray(W1, f32)
    W2 = np.asarray(W2, f32)
    br = np.asarray(br, f32)
    b1 = np.asarray(b1, f32)
    b2 = np.asarray(b2, f32)

    # shared (replicated) weight blocks
    w1a = np.ascontiguousarray(
        W1[:, :D, :].transpose(1, 0, 2).reshape(D, F)
        .reshape(NKC, 128, NFC, 128).transpose(2, 1, 0, 3)
        .reshape(NFC, 128, D)).astype(BF16)
    w1b = np.ascontiguousarray(
        W1[:, D:, :].transpose(1, 0, 2).reshape(D, F)
        .reshape(NKC, 128, NFC, 128).transpose(2, 1, 0, 3)
        .reshape(NFC, 128, D)).astype(BF16)
    w2 = np.ascontiguousarray(W2.reshape(F, D).reshape(NFC, 128, D)).astype(BF16)
    wr = np.ascontiguousarray(Wr.reshape(NKC, 128, K)).astype(BF16)
    br_bc = np.tile(np.broadcast_to(br[None, :], (128, K)), (1, NTC)).astype(f32)
    b1s = np.ascontiguousarray(b1.reshape(F).reshape(NFC, 128).T)
    b2s = b2.astype(BF16)
    identm = np.eye(128, dtype=f32)

    offs = np.array(OFF_ORDER, np.int64)
    in_maps = []
    outs_meta = []
    per_batch = L // (NCORES // B)          # 512 tokens, 4 shards per batch
    for c in range(NCORES):
        b = c // (NCORES // B)
        t0 = (c % (NCORES // B)) * per_batch
        # padded, transposed h slice  [D, TP]
        hpad = np.zeros((TP, D), f32)
        lo = t0 - PADL
        hi = t0 + T + PADL
        slo, shi = max(lo, 0), min(hi, L)
        hpad[slo - lo:shi - lo] = h_L[b, slo:shi]
        hTa = np.ascontiguousarray(hpad.T).astype(BF16)          # [D, TP]
        hTa = np.ascontiguousarray(hTa.reshape(NKC, 128, TP))

        # validity per (token, offset-order) -> [128, NTC*NOFF]
        tok = t0 + np.arange(T)
        nbr = tok[:, None] + offs[None, :]
        inb = (nbr >= 0) & (nbr < L)
        nbrc = np.clip(nbr, 0, L - 1)
        is_m = (mask[b] == 1)
        val = (inb & is_m[tok][:, None] & (~is_m[nbrc])).astype(f32)
        valT = np.ascontiguousarray(val.T)            # [NOFF, T]
        in_maps.append({
            "hT": hTa,
            "w1a": w1a, "w1b": w1b, "w2": w2, "wr": wr,
            "validT": valT,
            "ones10": np.ones((NOFF, 1), BF16),
            "br_bc": br_bc, "b1s": b1s, "b2s": b2s,
            "ident": identm,
        })
        outs_meta.append((b, t0))
    return in_maps, outs_meta


def kernel(**inputs):
    assert int(inputs["range_r"]) == R
    if "nc" not in _CACHE:
        _CACHE["nc"] = _build_graph()
    nc = _CACHE["nc"]
    in_maps, outs_meta = _prep_shards(
        inputs["h_L"], inputs["mask_flags"], inputs["Wr"], inputs["br"],
        inputs["W1"], inputs["b1"], inputs["W2"], inputs["b2"])
    res = run_bass_kernel_spmd(nc, in_maps, core_ids=list(range(NCORES)))
    out = np.zeros((B, L, D), np.float32)
    for c, (b, t0) in enumerate(outs_meta):
        out[b, t0:t0 + T] = res.results[c]["out"]
    return out
